# revision 7
# baseline (speedup 1.0000x reference)
"""MinGRU cell kernel for Trainium2 (8 NeuronCores, data-parallel over batch).

Reference computation (per sample n):
    zh = x[n] @ W.T + b            # (L, 2H)
    z, u = split(zh)               # each (L, H)
    s = sigmoid(z); a = 1 - s
    g = relu(u) + min(sigmoid(u), 0.5)      # == x+0.5 for x>=0, sigmoid(x) else
    h_t = a_t * h_{t-1} + s_t * g_t         # first-order linear recurrence

Device mapping (per core = one batch sample):
  - matmul on PE in fp16 (full rate, 1 col/cycle, FWL weight loads; fp8
    DoubleRow was measured 2x on PE but its e4m3 operand quantization noise
    puts the end-to-end error at 3.3e-2 > the 2e-2 gate, and any residual
    scheme costs as much as fp16)
  - epilogue batched over L-PAIRS (1024-wide ops amortize the ~300ns
    per-instruction overheads; batching over H-chunks is impossible because
    the ACT bias is per-partition)
  - sigmoid/relu epilogues on ACT (reads 2 PSUM banks per op), elementwise
    in fp16 on DVE (2x rate for 16-bit) + gpsimd, recurrence via the DVE
    scan op (state kept fp32 internally)
  - x / W / h all fp16 on the wire; transposes + casts on the host
"""

import sys
import numpy as np

if "/opt/trn_rl_repo" not in sys.path:
    sys.path.insert(0, "/opt/trn_rl_repo")

from contextlib import ExitStack

import concourse.bass as bass
import concourse.mybir as mybir
import concourse.tile as tile
from concourse import bass_utils
from concourse.bass_utils import run_bass_kernel_spmd

P = 128
N_CORES = 8
L = 4096
H = 1024
HIN = 1024
KC = HIN // P      # contraction chunks (8)
HC = H // P        # hidden chunks per half (8)
LT = 512           # L positions per matmul / scan (one PSUM bank)
LG = 2 * LT        # L positions per epilogue group (pair of banks)
NLG = L // LG      # 4 L-groups

F32 = mybir.dt.float32
F16 = mybir.dt.float16
AF = mybir.ActivationFunctionType
OP = mybir.AluOpType


def split_waits(nc, max_waits=1):
    """This walrus build only supports one sync wait per instruction; move
    extras onto preceding no-ops on the same engine."""
    for func in nc.m.functions:
        for b in func.blocks:
            idx = 0
            while idx < len(b.instructions):
                inst = b.instructions[idx]
                si = inst.sync_info
                if si is not None and len(si.on_wait) > max_waits:
                    waits = list(si.on_wait)
                    pre, keep = waits[:-max_waits], waits[-max_waits:]
                    pos = idx
                    while pre:
                        chunk, pre = pre[:max_waits], pre[max_waits:]
                        nop = mybir.InstNoOp(
                            name=nc.get_next_instruction_name(), ins=[], outs=[])
                        nop.engine = inst.engine
                        nop.sync_info = mybir.SyncInfo(on_wait=chunk, on_update=[])
                        nc.register_instruction(nop)
                        b.instructions.insert(pos, nop)
                        pos += 1
                        idx += 1
                    si.on_wait = keep
                idx += 1


def build_program():
    nc = bass.Bass()
    # xt[p, j, l] = x[l, 128j + p]; wt[p, j, o] = W[o, 128j + p]
    xt = nc.dram_tensor("xt", [P, KC, L], F16, kind="ExternalInput")
    wt = nc.dram_tensor("wt", [P, KC, 2 * H], F16, kind="ExternalInput")
    bz = nc.dram_tensor("bz", [P, HC], F32, kind="ExternalInput")
    bh = nc.dram_tensor("bh", [P, HC], F32, kind="ExternalInput")
    h0 = nc.dram_tensor("h0", [P, HC], F16, kind="ExternalInput")
    ht = nc.dram_tensor("ht", [H, L], F16, kind="ExternalOutput")

    with tile.TileContext(nc) as tc:
        with ExitStack() as ctx:
            pool = lambda name, bufs: ctx.enter_context(
                tc.tile_pool(name=name, bufs=bufs))
            w_pool = pool("w", 1)
            bias_pool = pool("bias", 1)
            xt_pool = pool("xt", 3)
            s_pool = pool("s", 3)
            a_pool = pool("a", 3)
            sg_pool = pool("sg", 3)
            u_pool = pool("u", 3)
            bv_pool = pool("bv", 3)
            h_pool = pool("h", 2)
            psum = ctx.enter_context(
                tc.tile_pool(name="psum", bufs=2, space="PSUM"))

            bz_sb = bias_pool.tile([P, HC], F32)
            nc.sync.dma_start(bz_sb[:], bz[:])
            bh_sb = bias_pool.tile([P, HC], F32)
            nc.sync.dma_start(bh_sb[:], bh[:])
            h0_sb = bias_pool.tile([P, HC], F16)
            nc.sync.dma_start(h0_sb[:], h0[:])

            def load_x(lg):
                tiles = []
                for ko in range(KC):
                    x_k = xt_pool.tile([P, LG], F16, tag=f"x{ko}")
                    nc.sync.dma_start(
                        x_k[:], xt[:, ko, lg * LG:(lg + 1) * LG])
                    tiles.append(x_k)
                return tiles

            # first x L-group interleaved with z-half weights, both split in
            # halves, so the first matmuls start after ~0.4 MiB of DMA; u
            # weights follow.
            x_first = []
            w_z, w_u = [], []
            for ko in range(KC):
                x_k = xt_pool.tile([P, LG], F16, tag=f"x{ko}")
                nc.sync.dma_start(x_k[:, 0:LT], xt[:, ko, 0:LT])
                x_first.append(x_k)
                wz_k = w_pool.tile([P, H], F16, tag=f"wz{ko}")
                nc.sync.dma_start(wz_k[:, 0:LT], wt[:, ko, 0:LT])
                nc.sync.dma_start(wz_k[:, LT:H], wt[:, ko, LT:H])
                w_z.append(wz_k)
            for ko in range(KC):
                nc.sync.dma_start(x_first[ko][:, LT:LG], xt[:, ko, LT:LG])
            for ko in range(KC):
                wu_k = w_pool.tile([P, H], F16, tag=f"wu{ko}")
                nc.sync.dma_start(wu_k[:, 0:LT], wt[:, ko, H:H + LT])
                nc.sync.dma_start(wu_k[:, LT:H], wt[:, ko, H + LT:2 * H])
                w_u.append(wu_k)

            def epilogue(c, lg, z_ps, u_ps):
                # batched elementwise over both banks (LG columns); on the
                # last L-group split the ACT ops per bank so the drain chain
                # is shorter.
                splits = 2 if lg == NLG - 1 else 1
                w = LG // splits
                sg_sb = sg_pool.tile([P, LG], F16, tag="sg")
                u_sb = u_pool.tile([P, LG], F16, tag="u")
                s_sb = s_pool.tile([P, LG], F16, tag="s")
                for si in range(splits):
                    sl = slice(si * w, (si + 1) * w)
                    psl = u_ps[:, si, :] if splits == 2 else u_ps[:, :, :]
                    zsl = z_ps[:, si, :] if splits == 2 else z_ps[:, :, :]
                    nc.scalar.activation(
                        sg_sb[:, sl], psl, AF.Sigmoid, bias=bh_sb[:, c:c + 1])
                    # r = relu(u + bias)
                    nc.scalar.activation(
                        u_sb[:, sl], psl, AF.Relu, bias=bh_sb[:, c:c + 1])
                    nc.scalar.activation(
                        s_sb[:, sl], zsl, AF.Sigmoid, bias=bz_sb[:, c:c + 1])

                # a = 1 - s
                a_sb = a_pool.tile([P, LG], F16, tag="a")
                nc.vector.tensor_scalar(
                    a_sb[:], s_sb[:], -1.0, 1.0, OP.mult, OP.add)
                # g = min(sigmoid(u), 0.5) + relu(u)
                nc.vector.scalar_tensor_tensor(
                    u_sb[:], sg_sb[:], 0.5, u_sb[:], OP.min, OP.add)
                # bval = s * g
                bv_sb = bv_pool.tile([P, LG], F16, tag="bv")
                nc.vector.tensor_tensor(
                    bv_sb[:], s_sb[:], u_sb[:], OP.mult)

                # two chained scans (the recurrence stays serial along L)
                prev = h_prev[c]
                for j in range(2):
                    h_sb = h_pool.tile([P, LT], F16, tag=f"h{c}{j}")
                    if prev is None:
                        init = h0_sb[:, c:c + 1]
                    else:
                        init = prev[:, LT - 1:LT]
                    nc.vector.tensor_tensor_scan(
                        h_sb[:], a_sb[:, j * LT:(j + 1) * LT],
                        bv_sb[:, j * LT:(j + 1) * LT], init,
                        OP.mult, OP.add)
                    nc.sync.dma_start(
                        ht[c * P:(c + 1) * P,
                           lg * LG + j * LT:lg * LG + (j + 1) * LT], h_sb[:])
                    prev = h_sb
                h_prev[c] = prev

            h_prev = [None] * HC
            for lg in range(NLG):
                x_sbs = x_first if lg == 0 else load_x(lg)

                for c in range(HC):
                    z_ps = psum.tile([P, 2, LT], F32, tag="zps")
                    u_ps = psum.tile([P, 2, LT], F32, tag="ups")
                    # ko outer, j inner: consecutive matmul pairs share the
                    # same stationary weights
                    for ko in range(KC):
                        for j in range(2):
                            nc.tensor.matmul(
                                z_ps[:, j, :], w_z[ko][:, c * P:(c + 1) * P],
                                x_sbs[ko][:, j * LT:(j + 1) * LT],
                                start=(ko == 0), stop=(ko == KC - 1))
                    for ko in range(KC):
                        for j in range(2):
                            nc.tensor.matmul(
                                u_ps[:, j, :], w_u[ko][:, c * P:(c + 1) * P],
                                x_sbs[ko][:, j * LT:(j + 1) * LT],
                                start=(ko == 0), stop=(ko == KC - 1))

                    epilogue(c, lg, z_ps, u_ps)

    split_waits(nc)
    return nc


_program_cache = {}


def _get_program():
    if "nc" not in _program_cache:
        _program_cache["nc"] = build_program()
    return _program_cache["nc"]


def _interleave16(a):
    """[rows, cols] fp32 -> [P, KC, cols] fp16 with rows i = 128j + p
    mapped to [p, j]."""
    q = a.astype(np.float16)
    rows, cols = q.shape
    return np.ascontiguousarray(
        q.reshape(rows // P, P, cols).transpose(1, 0, 2))


def make_in_maps(x, W, b, hx):
    """Host-side prep: cast to fp16 + lay out inputs for each core."""
    x = np.ascontiguousarray(x, dtype=np.float32)
    W = np.ascontiguousarray(W, dtype=np.float32)
    b = np.ascontiguousarray(b, dtype=np.float32)
    hx = np.ascontiguousarray(hx, dtype=np.float32)

    wt16 = _interleave16(np.ascontiguousarray(W.T))  # [P, KC, 2H]
    bz = np.ascontiguousarray(b[:H].reshape(HC, P).T)
    bh = np.ascontiguousarray(b[H:].reshape(HC, P).T)
    in_maps = []
    for n in range(x.shape[0]):
        xt16 = _interleave16(np.ascontiguousarray(x[n].T))
        in_maps.append({
            "xt": xt16,
            "wt": wt16,
            "bz": bz,
            "bh": bh,
            "h0": np.ascontiguousarray(
                hx[n].reshape(HC, P).T.astype(np.float16)),
        })
    return in_maps


def kernel(x, W, b, hx, _debug_result=None):
    N = x.shape[0]
    assert x.shape == (N_CORES, L, HIN) and W.shape == (2 * H, HIN)

    nc = _get_program()
    in_maps = make_in_maps(x, W, b, hx)

    res = run_bass_kernel_spmd(nc, in_maps, core_ids=list(range(N_CORES)))
    if _debug_result is not None:
        _debug_result.append(res)

    out = np.empty((N_CORES, L, H), np.float32)
    for n in range(N_CORES):
        out[n] = np.asarray(res.results[n]["ht"]).astype(np.float32).T
    return out


if __name__ == "__main__":
    rng = np.random.default_rng(0)
    x = rng.standard_normal((N_CORES, L, HIN), dtype=np.float32)
    W = rng.standard_normal((2 * H, HIN), dtype=np.float32) / np.sqrt(HIN)
    b = (rng.standard_normal(2 * H) * 0.01).astype(np.float32)
    hx = rng.random((N_CORES, H), dtype=np.float32)
    out = kernel(x, W, b, hx)
    print("ran ok", out.shape, out.dtype, float(np.abs(out).max()))
